# revision 10
# baseline (speedup 1.0000x reference)
"""Per-channel EMA (first-order linear recurrence along time) on 8 TRN2 cores.

  y[b, c, 0] = x[b, c, 0]
  y[b, c, t] = (1 - alpha[c]) * y[b, c, t-1] + alpha[c] * x[b, c, t]

Strategy (v3: radix-2 scan + Tensor-engine offload + fp16 IO + deinterleave)
  - Data-parallel over batch: B=32 -> 4 batches per core, alpha replicated.
  - Per core: 16 tiles of [128 channels (partitions), 2048 time (free)].
  - The DVE tensor_tensor_scan runs at ~2.1 cycles/element regardless of
    dtype, so a full-tile scan costs ~4.4us and 16 of them (~86us) dominated
    the v1 kernel. Here the recurrence is decimated by 2:
        even outputs:  z_m = y_{2m} = d^2 * z_{m-1} + u_m,
                       u_m = a*d*x_{2m-1} + a*x_{2m}   (u_0 = x_0)
        odd outputs:   y_{2m+1} = d * z_m + a * x_{2m+1}
    The DVE scans only the 1024 even columns (~2.3us/tile); u and the odd
    reconstruction are diagonal matmuls on the otherwise-idle Tensor engine
    (per-channel scale = diag weight matrix, fp16, PSUM f32 accumulation):
        u    = diag(a*d) @ x_odd<<1 + diag(a) @ x_even  (+ diag(d) @ x_0 on
               column 0, making u_0 = (a+d)*x_0 = x_0 exactly)
        y_od = diag(d) @ z + diag(a) @ x_odd
    The ACT engine copies the odd PSUM back to SBUF. Diag weights (fp16) and
    d^2 (fp32) are precomputed on host from alpha - alpha is a kernel input.
  - The host de-interleaves x (and re-interleaves y) into [.., 2, L/2]
    even/odd halves, so every device-side access pattern is contiguous:
    strided fp16 reads cost the PE ~3x its contiguous rate.
  - HBM IO is fp16 (host converts): halves the 32 MiB/core round trip to
    16 MiB (~47us roofline at 358 GB/s/core). The scan state stays fp32
    internally (hardware guarantee) with d^2 in fp32, so the recurrence does
    not accumulate quantization error (|d|<1 contraction; measured rel err
    ~5e-4, gate is 2e-2).
  - Queue discipline: x loads alone on the SP HWDGE queue (weights/d2 ride
    the ACT ring); PE emitted with a one-tile skew (BCA_{n+1} before DE_n)
    so the Tensor engine pipelines around the scan; ACT carries the odd
    copies; stores ride SWDGE on the idle GpSimd queue except the last two
    tiles, which use the ACT HWDGE ring to avoid the slow SWDGE tail drain.
  - Tile 0 is processed in four chained quarter-chunks so the first scan
    starts as soon as the first 128 KiB lands; the last tile is processed in
    two chained halves so its reconstruction/copy/store overlap its second
    half's scan, pulling the kernel-drain point forward.
"""

import numpy as np

import concourse.bass as bass
import concourse.bacc as bacc
import concourse.mybir as mybir
from concourse.tile import TileContext
from concourse.bass_utils import run_bass_kernel_spmd

B, C, L = 32, 512, 2048
N_CORES = 8
B_SH = B // N_CORES  # 4 batches per core
P = 128              # SBUF partitions
N_CB = C // P        # 4 channel blocks
N_TILES = B_SH * N_CB
LH = L // 2          # 1024 scan columns per tile

_F32 = mybir.dt.float32
_F16 = mybir.dt.float16

mult = mybir.AluOpType.mult
add = mybir.AluOpType.add


def build_nc() -> bass.Bass:
    # Bacc (not raw Bass): its compile() runs generate_event_semaphores,
    # which splits multi-sem waits — TRN2 allows at most one wait command
    # per instruction, and Tile freely emits several.
    nc = bacc.Bacc()
    # x/y are host-deinterleaved: [.., 0:LH] = even time steps, [.., LH:L]
    # = odd time steps.
    x = nc.dram_tensor("x", [B_SH, C, L], _F16, kind="ExternalInput")
    # w[p, (cb*3+j)*P + m]: diag weight blocks, j=0: diag(a), 1: diag(a*d),
    # 2: diag(d) for channel block cb (built on host, already in SBUF layout)
    w = nc.dram_tensor("w", [P, N_CB * 3 * P], _F16, kind="ExternalInput")
    d2 = nc.dram_tensor("d2", [1, C], _F32, kind="ExternalInput")
    y = nc.dram_tensor("y", [B_SH, C, L], _F16, kind="ExternalOutput")

    with TileContext(nc) as tc:
        with (
            tc.tile_pool(name="xp", bufs=6) as xp,
            tc.tile_pool(name="yp", bufs=6) as yp,
            tc.tile_pool(name="cp", bufs=1) as cp,
            tc.tile_pool(name="up", bufs=2, space="PSUM") as up,
            tc.tile_pool(name="wp", bufs=2, space="PSUM") as wp,
        ):
            # consts ride the ACT HWDGE ring so x loads own the SP queue
            wt = cp.tile([P, N_CB * 3 * P], _F16, tag="wt", name="wt")
            nc.scalar.dma_start(out=wt, in_=w[:, :])
            d2t = cp.tile([P, N_CB], _F32, tag="d2t", name="d2t")
            nc.scalar.dma_start(
                out=d2t, in_=d2[0].rearrange("(j p) -> p j", j=N_CB)
            )
            # warm-up ACT op: pulls the activation-table load off the first
            # odd-copy's critical path (depends only on the tiny d2 load)
            warm = cp.tile([P, N_CB], _F32, tag="warm", name="warm")
            nc.scalar.mul(warm, d2t, 1.0)

            def W(cb, j):
                o = (cb * 3 + j) * P
                return wt[:, o : o + P]

            tiles = []  # (xt, yt, cb, b) in emission order

            def emit_load(n, chunks=1):
                cb, b = divmod(n, B_SH)
                cs = slice(cb * P, (cb + 1) * P)
                xt = xp.tile([P, L], _F16, tag="x", name="xt")
                yt = yp.tile([P, L], _F16, tag="y", name="yt")
                cw = LH // chunks
                for c in range(chunks):
                    lo, hi = c * cw, (c + 1) * cw
                    nc.sync.dma_start(out=xt[:, lo:hi], in_=x[b, cs, lo:hi])
                    nc.sync.dma_start(
                        out=xt[:, LH + lo : LH + hi],
                        in_=x[b, cs, LH + lo : LH + hi],
                    )
                tiles.append((xt, yt, cb, b))

            def emit_bca(n, lo, hi):
                """u[lo:hi] = diag(a)@x_ev[lo:hi] + diag(ad)@x_od[lo-1:hi-1]
                (+ diag(d)@x_0 on column 0), emitted as <=512-col matmuls."""
                xt, yt, cb, b = tiles[n]
                u = tiles_u[n]
                Wa, Wad, Wd = W(cb, 0), W(cb, 1), W(cb, 2)
                for s in range(lo, hi, 512):
                    e = min(s + 512, hi)
                    nc.tensor.matmul(
                        out=u[:, s:e], lhsT=Wa, rhs=xt[:, s:e],
                        start=True, stop=False,
                    )
                for s in range(lo, hi, 512):
                    e = min(s + 512, hi)
                    s2 = max(s, 1)  # u_0 has no x_{-1} term
                    # stop on the final writer of each PSUM region: regions
                    # containing column 0 are finished by the A-matmul below
                    nc.tensor.matmul(
                        out=u[:, s2:e], lhsT=Wad,
                        rhs=xt[:, LH + s2 - 1 : LH + e - 1],
                        start=False, stop=(s > 0),
                    )
                if lo == 0:
                    nc.tensor.matmul(
                        out=u[:, 0:1], lhsT=Wd, rhs=xt[:, 0:1],
                        start=False, stop=True,
                    )

            def emit_scan(n, lo, hi):
                xt, yt, cb, b = tiles[n]
                nc.vector.tensor_tensor_scan(
                    out=yt[:, lo:hi],
                    data0=d2t[:, cb : cb + 1].broadcast_to([P, hi - lo]),
                    data1=tiles_u[n][:, lo:hi],
                    initial=0.0 if lo == 0 else yt[:, lo - 1 : lo],
                    op0=mult,
                    op1=add,
                )

            def emit_de(n, lo, hi):
                """wv[lo:hi] = diag(d) @ z[lo:hi] + diag(a) @ x_od[lo:hi]"""
                xt, yt, cb, b = tiles[n]
                wv = tiles_w[n]
                Wa, Wd = W(cb, 0), W(cb, 2)
                for s in range(lo, hi, 512):
                    e = min(s + 512, hi)
                    nc.tensor.matmul(
                        out=wv[:, s:e], lhsT=Wd, rhs=yt[:, s:e],
                        start=True, stop=False,
                    )
                for s in range(lo, hi, 512):
                    e = min(s + 512, hi)
                    nc.tensor.matmul(
                        out=wv[:, s:e], lhsT=Wa,
                        rhs=xt[:, LH + s : LH + e],
                        start=False, stop=True,
                    )

            def emit_copy(n, lo, hi):
                xt, yt, cb, b = tiles[n]
                nc.scalar.copy(yt[:, LH + lo : LH + hi], tiles_w[n][:, lo:hi])

            def emit_store(n, lo, hi, ring):
                xt, yt, cb, b = tiles[n]
                cs = slice(cb * P, (cb + 1) * P)
                if lo == 0 and hi == LH:
                    dma = nc.scalar.dma_start if ring else nc.gpsimd.dma_start
                    dma(out=y[b, cs, :], in_=yt)
                else:
                    dma = nc.scalar.dma_start if ring else nc.gpsimd.dma_start
                    dma(out=y[b, cs, lo:hi], in_=yt[:, lo:hi])
                    dma(
                        out=y[b, cs, LH + lo : LH + hi],
                        in_=yt[:, LH + lo : LH + hi],
                    )

            tiles_u = []
            tiles_w = []

            def alloc_psum(n):
                tiles_u.append(up.tile([P, LH], _F32, tag="u", name="u"))
                tiles_w.append(wp.tile([P, LH], _F32, tag="w", name="wv"))

            LAST = N_TILES - 1

            # tile 0: four chained quarter-chunks for fast pipeline start
            emit_load(0, chunks=4)
            alloc_psum(0)
            for c in range(4):
                lo, hi = c * (LH // 4), (c + 1) * (LH // 4)
                emit_bca(0, lo, hi)
                emit_scan(0, lo, hi)

            for n in range(1, LAST):
                emit_load(n)
                alloc_psum(n)
                emit_bca(n, 0, LH)
                emit_de(n - 1, 0, LH)
                emit_scan(n, 0, LH)
                emit_copy(n - 1, 0, LH)
                emit_store(n - 1, 0, LH, ring=(n - 1 >= N_TILES - 2))

            # last tile: two chained halves so DE/copy/store of the first
            # half overlap the second half's scan
            emit_load(LAST)
            alloc_psum(LAST)
            emit_bca(LAST, 0, LH)
            emit_de(LAST - 1, 0, LH)
            emit_scan(LAST, 0, LH // 2)
            emit_scan(LAST, LH // 2, LH)
            emit_copy(LAST - 1, 0, LH)
            emit_store(LAST - 1, 0, LH, ring=True)
            emit_de(LAST, 0, LH // 2)
            emit_copy(LAST, 0, LH // 2)
            emit_store(LAST, 0, LH // 2, ring=True)
            emit_de(LAST, LH // 2, LH)
            emit_copy(LAST, LH // 2, LH)
            emit_store(LAST, LH // 2, LH, ring=True)

    nc.compile()
    return nc


def _host_consts(alpha: np.ndarray):
    """Diag weight blocks (fp16, SBUF layout) + d^2 (fp32) from alpha."""
    a = alpha[0].astype(np.float64)  # [C]
    d = 1.0 - a
    # fp16 diag entries; d16 = 1 - a16 in fp16 arithmetic so the u_0 column
    # fixup (a16 + d16) lands as close to exactly 1 as fp16 allows
    a16 = a.astype(np.float16)
    d16 = (np.float16(1.0) - a16).astype(np.float16)
    ad16 = (a16 * d16).astype(np.float16)
    w = np.zeros((P, N_CB * 3 * P), dtype=np.float16)
    idx = np.arange(P)
    for cb in range(N_CB):
        s = slice(cb * P, (cb + 1) * P)
        for j, v in enumerate((a16[s], ad16[s], d16[s])):
            w[idx, (cb * 3 + j) * P + idx] = v
    d2 = (d * d).astype(np.float32)[None, :]  # [1, C]
    return w, d2


def _deinterleave(x: np.ndarray) -> np.ndarray:
    """[.., L] f -> [.., L] fp16 with [.., 0:LH]=evens, [.., LH:L]=odds."""
    out = np.empty(x.shape, dtype=np.float16)
    out[..., 0:LH] = x[..., 0::2]
    out[..., LH:L] = x[..., 1::2]
    return out


def _reinterleave(y: np.ndarray) -> np.ndarray:
    out = np.empty(y.shape, dtype=np.float32)
    out[..., 0::2] = y[..., 0:LH]
    out[..., 1::2] = y[..., LH:L]
    return out


_cached_nc = None


def _get_nc() -> bass.Bass:
    global _cached_nc
    if _cached_nc is None:
        _cached_nc = build_nc()
    return _cached_nc


def kernel(x: np.ndarray, alpha: np.ndarray) -> np.ndarray:
    assert x.shape == (B, C, L) and alpha.shape == (1, C)
    xd = _deinterleave(np.asarray(x, dtype=np.float32))
    alpha = np.ascontiguousarray(alpha, dtype=np.float32)
    w, d2 = _host_consts(alpha)
    nc = _get_nc()
    in_maps = [
        {"x": xd[c * B_SH : (c + 1) * B_SH], "w": w, "d2": d2}
        for c in range(N_CORES)
    ]
    res = run_bass_kernel_spmd(nc, in_maps, list(range(N_CORES)))
    return np.concatenate(
        [_reinterleave(r["y"]) for r in res.results], axis=0
    )


# revision 12
# speedup vs baseline: 1.0587x; 1.0587x over previous
"""Per-channel EMA (first-order linear recurrence along time) on 8 TRN2 cores.

  y[b, c, 0] = x[b, c, 0]
  y[b, c, t] = (1 - alpha[c]) * y[b, c, t-1] + alpha[c] * x[b, c, t]

Strategy (v3: radix-2 scan + Tensor-engine offload + fp16 IO + deinterleave)
  - Data-parallel over batch: B=32 -> 4 batches per core, alpha replicated.
  - Per core: 16 tiles of [128 channels (partitions), 2048 time (free)].
  - The DVE tensor_tensor_scan runs at ~2.1 cycles/element regardless of
    dtype, so a full-tile scan costs ~4.4us and 16 of them (~86us) dominated
    the v1 kernel. Here the recurrence is decimated by 2:
        even outputs:  z_m = y_{2m} = d^2 * z_{m-1} + u_m,
                       u_m = a*d*x_{2m-1} + a*x_{2m}   (u_0 = x_0)
        odd outputs:   y_{2m+1} = d * z_m + a * x_{2m+1}
    The DVE scans only the 1024 even columns (~2.3us/tile); u and the odd
    reconstruction are diagonal matmuls on the otherwise-idle Tensor engine
    (per-channel scale = diag weight matrix, fp16, PSUM f32 accumulation):
        u    = diag(a*d) @ x_odd<<1 + diag(a) @ x_even  (+ diag(d) @ x_0 on
               column 0, making u_0 = (a+d)*x_0 = x_0 exactly)
        y_od = diag(d) @ z + diag(a) @ x_odd
    The ACT engine copies the odd PSUM back to SBUF. Diag weights (fp16) and
    d^2 (fp32) are precomputed on host from alpha - alpha is a kernel input.
  - The host de-interleaves x (and re-interleaves y) into [.., 2, L/2]
    even/odd halves, so every device-side access pattern is contiguous:
    strided fp16 reads cost the PE ~3x its contiguous rate.
  - HBM IO is fp16 (host converts): halves the 32 MiB/core round trip to
    16 MiB (~47us roofline at 358 GB/s/core). The scan state stays fp32
    internally (hardware guarantee) with d^2 in fp32, so the recurrence does
    not accumulate quantization error (|d|<1 contraction; measured rel err
    ~5e-4, gate is 2e-2).
  - Queue discipline: x loads alone on the SP HWDGE queue (weights/d2 ride
    the ACT ring); PE emitted with a one-tile skew (BCA_{n+1} before DE_n)
    so the Tensor engine pipelines around the scan; ACT carries the odd
    copies; stores ride SWDGE on the idle GpSimd queue except the last two
    tiles, which use the ACT HWDGE ring to avoid the slow SWDGE tail drain.
  - Tile 0 is processed in four chained quarter-chunks so the first scan
    starts as soon as the first 128 KiB lands; the last tile is processed in
    two chained halves so its reconstruction/copy/store overlap its second
    half's scan, pulling the kernel-drain point forward.
"""

import os

import numpy as np

import concourse.bass as bass
import concourse.bacc as bacc
import concourse.mybir as mybir
from concourse.tile import TileContext
from concourse.bass_utils import run_bass_kernel_spmd

B, C, L = 32, 512, 2048
N_CORES = 8
B_SH = B // N_CORES  # 4 batches per core
P = 128              # SBUF partitions
N_CB = C // P        # 4 channel blocks
N_TILES = B_SH * N_CB
LH = L // 2          # 1024 scan columns per tile

_F32 = mybir.dt.float32
_F16 = mybir.dt.float16

mult = mybir.AluOpType.mult
add = mybir.AluOpType.add


def build_nc() -> bass.Bass:
    # Bacc (not raw Bass): its compile() runs generate_event_semaphores,
    # which splits multi-sem waits — TRN2 allows at most one wait command
    # per instruction, and Tile freely emits several.
    nc = bacc.Bacc()
    # x/y are host-deinterleaved: [.., 0:LH] = even time steps, [.., LH:L]
    # = odd time steps.
    x = nc.dram_tensor("x", [B_SH, C, L], _F16, kind="ExternalInput")
    # w[p, (cb*3+j)*P + m]: diag weight blocks, j=0: diag(a), 1: diag(a*d),
    # 2: diag(d) for channel block cb (built on host, already in SBUF layout)
    w = nc.dram_tensor("w", [P, N_CB * 3 * P], _F16, kind="ExternalInput")
    d2 = nc.dram_tensor("d2", [1, C], _F32, kind="ExternalInput")
    y = nc.dram_tensor("y", [B_SH, C, L], _F16, kind="ExternalOutput")

    with TileContext(nc) as tc:
        with (
            tc.tile_pool(name="xp", bufs=6) as xp,
            tc.tile_pool(name="yp", bufs=6) as yp,
            tc.tile_pool(name="cp", bufs=1) as cp,
            tc.tile_pool(name="up", bufs=2, space="PSUM") as up,
            tc.tile_pool(name="wp", bufs=2, space="PSUM") as wp,
        ):
            # consts ride the ACT HWDGE ring so x loads own the SP queue
            wt = cp.tile([P, N_CB * 3 * P], _F16, tag="wt", name="wt")
            nc.scalar.dma_start(out=wt, in_=w[:, :])
            d2t = cp.tile([P, N_CB], _F32, tag="d2t", name="d2t")
            nc.scalar.dma_start(
                out=d2t, in_=d2[0].rearrange("(j p) -> p j", j=N_CB)
            )
            # warm-up ACT op: pulls the activation-table load off the first
            # odd-copy's critical path (depends only on the tiny d2 load)
            warm = cp.tile([P, N_CB], _F32, tag="warm", name="warm")
            nc.scalar.mul(warm, d2t, 1.0)

            def W(cb, j):
                o = (cb * 3 + j) * P
                return wt[:, o : o + P]

            tiles = []  # (xt, yt, cb, b) in emission order

            def emit_load(n, chunks=1):
                cb, b = divmod(n, B_SH)
                cs = slice(cb * P, (cb + 1) * P)
                xt = xp.tile([P, L], _F16, tag="x", name="xt")
                yt = yp.tile([P, L], _F16, tag="y", name="yt")
                cw = LH // chunks
                for c in range(chunks):
                    lo, hi = c * cw, (c + 1) * cw
                    nc.sync.dma_start(out=xt[:, lo:hi], in_=x[b, cs, lo:hi])
                    nc.sync.dma_start(
                        out=xt[:, LH + lo : LH + hi],
                        in_=x[b, cs, LH + lo : LH + hi],
                    )
                tiles.append((xt, yt, cb, b))

            def emit_bca(n, lo, hi):
                """u[lo:hi] = diag(a)@x_ev[lo:hi] + diag(ad)@x_od[lo-1:hi-1]
                (+ diag(d)@x_0 on column 0), emitted as <=512-col matmuls."""
                xt, yt, cb, b = tiles[n]
                u = tiles_u[n]
                Wa, Wad, Wd = W(cb, 0), W(cb, 1), W(cb, 2)
                for s in range(lo, hi, 512):
                    e = min(s + 512, hi)
                    nc.tensor.matmul(
                        out=u[:, s:e], lhsT=Wa, rhs=xt[:, s:e],
                        start=True, stop=False,
                    )
                for s in range(lo, hi, 512):
                    e = min(s + 512, hi)
                    s2 = max(s, 1)  # u_0 has no x_{-1} term
                    # stop on the final writer of each PSUM region: regions
                    # containing column 0 are finished by the A-matmul below
                    nc.tensor.matmul(
                        out=u[:, s2:e], lhsT=Wad,
                        rhs=xt[:, LH + s2 - 1 : LH + e - 1],
                        start=False, stop=(s > 0),
                    )
                if lo == 0:
                    nc.tensor.matmul(
                        out=u[:, 0:1], lhsT=Wd, rhs=xt[:, 0:1],
                        start=False, stop=True,
                    )

            def emit_scan(n, lo, hi):
                xt, yt, cb, b = tiles[n]
                nc.vector.tensor_tensor_scan(
                    out=yt[:, lo:hi],
                    data0=d2t[:, cb : cb + 1].broadcast_to([P, hi - lo]),
                    data1=tiles_u[n][:, lo:hi],
                    initial=0.0 if lo == 0 else yt[:, lo - 1 : lo],
                    op0=mult,
                    op1=add,
                )

            def emit_de(n, lo, hi):
                """wv[lo:hi] = diag(d) @ z[lo:hi] + diag(a) @ x_od[lo:hi]"""
                xt, yt, cb, b = tiles[n]
                wv = tiles_w[n]
                Wa, Wd = W(cb, 0), W(cb, 2)
                for s in range(lo, hi, 512):
                    e = min(s + 512, hi)
                    nc.tensor.matmul(
                        out=wv[:, s:e], lhsT=Wd, rhs=yt[:, s:e],
                        start=True, stop=False,
                    )
                for s in range(lo, hi, 512):
                    e = min(s + 512, hi)
                    nc.tensor.matmul(
                        out=wv[:, s:e], lhsT=Wa,
                        rhs=xt[:, LH + s : LH + e],
                        start=False, stop=True,
                    )

            def emit_copy(n, lo, hi):
                xt, yt, cb, b = tiles[n]
                nc.scalar.copy(yt[:, LH + lo : LH + hi], tiles_w[n][:, lo:hi])

            def emit_store(n, lo, hi, ring):
                xt, yt, cb, b = tiles[n]
                cs = slice(cb * P, (cb + 1) * P)
                if lo == 0 and hi == LH:
                    dma = nc.scalar.dma_start if ring else nc.gpsimd.dma_start
                    dma(out=y[b, cs, :], in_=yt)
                else:
                    dma = nc.scalar.dma_start if ring else nc.gpsimd.dma_start
                    dma(out=y[b, cs, lo:hi], in_=yt[:, lo:hi])
                    dma(
                        out=y[b, cs, LH + lo : LH + hi],
                        in_=yt[:, LH + lo : LH + hi],
                    )

            tiles_u = []
            tiles_w = []

            def alloc_psum(n):
                tiles_u.append(up.tile([P, LH], _F32, tag="u", name="u"))
                tiles_w.append(wp.tile([P, LH], _F32, tag="w", name="wv"))

            LAST = N_TILES - 1

            # tile 0: chained quarter-chunks for fast pipeline start
            T0_CHUNKS = int(os.environ.get("T0_CHUNKS", "4"))
            emit_load(0, chunks=T0_CHUNKS)
            alloc_psum(0)
            for c in range(T0_CHUNKS):
                lo, hi = c * (LH // T0_CHUNKS), (c + 1) * (LH // T0_CHUNKS)
                emit_bca(0, lo, hi)
                emit_scan(0, lo, hi)

            for n in range(1, LAST):
                emit_load(n)
                alloc_psum(n)
                emit_bca(n, 0, LH)
                emit_de(n - 1, 0, LH)
                emit_scan(n, 0, LH)
                emit_copy(n - 1, 0, LH)
                emit_store(n - 1, 0, LH, ring=(n - 1 >= N_TILES - 2))

            # last tile: two chained halves so DE/copy/store of the first
            # half overlap the second half's scan
            emit_load(LAST)
            alloc_psum(LAST)
            emit_bca(LAST, 0, LH)
            emit_de(LAST - 1, 0, LH)
            if os.environ.get("TL_SPLIT", "1") == "1":
                emit_scan(LAST, 0, LH // 2)
                emit_scan(LAST, LH // 2, LH)
                emit_copy(LAST - 1, 0, LH)
                emit_store(LAST - 1, 0, LH, ring=True)
                emit_de(LAST, 0, LH // 2)
                emit_copy(LAST, 0, LH // 2)
                emit_store(LAST, 0, LH // 2, ring=True)
                emit_de(LAST, LH // 2, LH)
                emit_copy(LAST, LH // 2, LH)
                emit_store(LAST, LH // 2, LH, ring=True)
            else:
                emit_scan(LAST, 0, LH)
                emit_copy(LAST - 1, 0, LH)
                emit_store(LAST - 1, 0, LH, ring=True)
                emit_de(LAST, 0, LH)
                emit_copy(LAST, 0, LH)
                emit_store(LAST, 0, LH, ring=True)

    nc.compile()
    return nc


def _host_consts(alpha: np.ndarray):
    """Diag weight blocks (fp16, SBUF layout) + d^2 (fp32) from alpha."""
    a = alpha[0].astype(np.float64)  # [C]
    d = 1.0 - a
    # fp16 diag entries; d16 = 1 - a16 in fp16 arithmetic so the u_0 column
    # fixup (a16 + d16) lands as close to exactly 1 as fp16 allows
    a16 = a.astype(np.float16)
    d16 = (np.float16(1.0) - a16).astype(np.float16)
    ad16 = (a16 * d16).astype(np.float16)
    w = np.zeros((P, N_CB * 3 * P), dtype=np.float16)
    idx = np.arange(P)
    for cb in range(N_CB):
        s = slice(cb * P, (cb + 1) * P)
        for j, v in enumerate((a16[s], ad16[s], d16[s])):
            w[idx, (cb * 3 + j) * P + idx] = v
    d2 = (d * d).astype(np.float32)[None, :]  # [1, C]
    return w, d2


def _deinterleave(x: np.ndarray) -> np.ndarray:
    """[.., L] f -> [.., L] fp16 with [.., 0:LH]=evens, [.., LH:L]=odds."""
    out = np.empty(x.shape, dtype=np.float16)
    out[..., 0:LH] = x[..., 0::2]
    out[..., LH:L] = x[..., 1::2]
    return out


def _reinterleave(y: np.ndarray) -> np.ndarray:
    out = np.empty(y.shape, dtype=np.float32)
    out[..., 0::2] = y[..., 0:LH]
    out[..., 1::2] = y[..., LH:L]
    return out


_cached_nc = None


def _get_nc() -> bass.Bass:
    global _cached_nc
    if _cached_nc is None:
        _cached_nc = build_nc()
    return _cached_nc


def run(x: np.ndarray, alpha: np.ndarray, nc=None, **spmd_kwargs):
    """Full host path: prep inputs, run on 8 cores, reassemble output.
    Returns (y, BassKernelResults)."""
    assert x.shape == (B, C, L) and alpha.shape == (1, C)
    xd = _deinterleave(np.asarray(x, dtype=np.float32))
    alpha = np.ascontiguousarray(alpha, dtype=np.float32)
    w, d2 = _host_consts(alpha)
    if nc is None:
        nc = _get_nc()
    in_maps = [
        {"x": xd[c * B_SH : (c + 1) * B_SH], "w": w, "d2": d2}
        for c in range(N_CORES)
    ]
    res = run_bass_kernel_spmd(nc, in_maps, list(range(N_CORES)), **spmd_kwargs)
    y = np.concatenate([_reinterleave(r["y"]) for r in res.results], axis=0)
    return y, res


def kernel(x: np.ndarray, alpha: np.ndarray) -> np.ndarray:
    return run(x, alpha)[0]
